# revision 2
# baseline (speedup 1.0000x reference)
"""Trainium2 Bass kernel for nn_DendriticCompartment (dense_mlp).

reference math:
    gates = sigmoid(x @ W_gate.T + b_gate)                      # (B, 4)
    seg_s = x @ W_seg[s].T + b_seg[s]                           # (B, 2048) per s
    plateau_s = sigmoid(5*(seg_s - thr_s))
    stacked_s = seg_s * plateau_s * gates[:, s:s+1]
    out = sum_s stacked_s + 0.1*sign(prod_s stacked_s)*prod_s|stacked_s|^(1/4)

Sharding: data-parallel on batch B (8192 -> 1024 cols per core); every core
computes all 4 segments and the full d_state for its batch slice.  Everything
is computed TRANSPOSED on device: out^T[d, b] tiles with d on partitions and
batch on the free dim; x is pre-transposed on the host so the moving matmul
operand loads with contiguous DMA and the PE never runs a transpose.  Matmul
inputs are bf16 (rel err 7.9e-3 vs the 2e-2 gate; fp32 PSUM accumulation);
per-core tensor roofline 8192*2048*1024 MACs / (128*128/cyc) / 2.4GHz = 437us.

Epilogue design (cost-model-trace driven; the scalar/vector/pool engines must
all stay well under the PE's ~28us per dh block):
- ACT runs ONLY sigmoid-set functions (sigmoid + copy), so the act table is
  loaded once: sign|q|^(1/4) is computed on DVE as a uint16 exponent-shift
  magic (y = ((|q|_bits >> 2) + 12188) | signbit, max rel err 3.9% on a term
  that is <= 0.35 of the output) instead of sign+sqrt+sqrt on ACT, which cost
  2 LoadActFuncSet (~1.3us each) per dh block.
- All elementwise intermediates are bf16 (2x DVE modes, halved traffic); the
  final 0.1^4 is folded into the product chain; output is stored bf16 and
  upcast on the host.
- Pool (gpsimd) fp32 tensor ops cost ~2.1us/op, so Pool does only the 4 gate
  muls per block; the sum/product tree runs on DVE.
- All DMA kickoffs ride HWDGE queues (sync + scalar), never SWDGE on Pool;
  kickoff order is tuned so the first x quarter lands ASAP and w0 lands just
  as the gates GEMM drains; w blocks prefetch 3 deep on the scalar queue.
- PSUM: 3 main acc buffers (PE runs up to 3 segment GEMMs ahead); the gates
  accumulate + broadcast rotate through the same ring at rep start.
- The gate broadcast matmuls are emitted after the first main GEMM so the PE
  does not stall on the gates sigmoid; the last dh block's epilogue runs in
  two 512-col halves to cut the end-of-kernel drain.
"""
import numpy as np
import ml_dtypes
from contextlib import ExitStack

import concourse.bass as bass
import concourse.mybir as mybir
import concourse.tile as tile
from concourse import bacc
from concourse.bass_utils import run_bass_kernel_spmd

FP32 = mybir.dt.float32
BF16 = mybir.dt.bfloat16
U16 = mybir.dt.uint16
AF = mybir.ActivationFunctionType
ALU = mybir.AluOpType
NPBF = ml_dtypes.bfloat16

B, D_IN, D_STATE, NSEG = 8192, 2048, 2048, 4
NCORES = 8
BSL = B // NCORES                  # 1024 batch columns per core
P = 128
KT = D_IN // P                     # 16 contraction tiles
NDH = D_STATE // P                 # 16 d-blocks of the output
NDB = NSEG * NDH                   # 64 weight blocks (dh-major, then s)
NQ = 4                             # x^T quarter tiles per rep
JQ = KT // NQ                      # 4 k-tiles per quarter
H = 512                            # psum bank width in fp32
MAGIC = 12188                      # bf16 bits: |q|^(1/4) ~ (bits>>2)+MAGIC


def build_nc(repeats=1):
    nc = bacc.Bacc("TRN2", debug=False, target_bir_lowering=False,
                   num_devices=NCORES)

    xt_d = nc.dram_tensor("xt", [D_IN, BSL], BF16, kind="ExternalInput").ap()
    w_d = nc.dram_tensor("w", [NDB, P, KT, P], BF16, kind="ExternalInput").ap()
    wg_d = nc.dram_tensor("wg", [P, KT * NSEG], BF16, kind="ExternalInput").ap()
    bg_d = nc.dram_tensor("bg", [NSEG], FP32, kind="ExternalInput").ap()
    sel_d = nc.dram_tensor("sel", [NSEG, NSEG * P], BF16, kind="ExternalInput").ap()
    cb_d = nc.dram_tensor("cb", [P, 2 * NDB], FP32, kind="ExternalInput").ap()
    out_d = nc.dram_tensor("out", [D_STATE, BSL], BF16, kind="ExternalOutput").ap()

    with tile.TileContext(nc) as tc, ExitStack() as ctx:
        const = ctx.enter_context(tc.tile_pool(name="const", bufs=1))
        xt_p = ctx.enter_context(tc.tile_pool(name="xt", bufs=2))
        w_p = ctx.enter_context(tc.tile_pool(name="w", bufs=6))
        gbc_p = ctx.enter_context(tc.tile_pool(name="gbc", bufs=1))
        pl_p = ctx.enter_context(tc.tile_pool(name="pl", bufs=2))
        u_p = ctx.enter_context(tc.tile_pool(name="u", bufs=2))
        st_p = ctx.enter_context(tc.tile_pool(name="st", bufs=2))
        mx_p = ctx.enter_context(tc.tile_pool(name="mx", bufs=2))
        ps_m = ctx.enter_context(tc.tile_pool(name="ps_m", bufs=3, space="PSUM"))

        xt_r = xt_d.rearrange("(q j p) b -> q p j b", p=P, j=JQ)
        out_r = out_d.rearrange("(dh p) b -> dh p b", p=P)

        # kickoff order tuned for the serial DMA path: first x quarter,
        # tiny consts, remaining quarters, then the first weight blocks --
        # so the gates GEMM starts ASAP and w0 lands right when the main
        # loop needs it
        xq0 = []
        xqt = xt_p.tile([P, JQ, BSL], BF16, tag="xq0", name="xq0_0")
        nc.sync.dma_start(out=xqt[:], in_=xt_r[0])
        xq0.append(xqt)
        wgt = const.tile([P, KT, NSEG], BF16)
        nc.sync.dma_start(
            out=wgt[:], in_=wg_d.rearrange("p (kt s) -> p kt s", s=NSEG))
        selt = const.tile([NSEG, NSEG * P], BF16)
        nc.sync.dma_start(out=selt[:], in_=sel_d)
        bgp = const.tile([NSEG, 1], FP32)
        nc.sync.dma_start(
            out=bgp[:],
            in_=bass.AP(tensor=bg_d.tensor, offset=bg_d.offset,
                        ap=[[1, NSEG], [1, 1]]))
        cbt = const.tile([P, 2 * NDB], FP32)
        nc.sync.dma_start(out=cbt[:], in_=cb_d)
        for q in range(1, NQ):
            xqt = xt_p.tile([P, JQ, BSL], BF16, tag=f"xq{q}", name=f"xq0_{q}")
            nc.sync.dma_start(out=xqt[:], in_=xt_r[q])
            xq0.append(xqt)

        for rep in range(repeats):
            # ---- load x^T slice (rep 0 preloaded above) ----
            if rep == 0:
                xq = xq0
            else:
                xq = []
                for q in range(NQ):
                    xqt = xt_p.tile([P, JQ, BSL], BF16, tag=f"xq{q}")
                    nc.sync.dma_start(out=xqt[:], in_=xt_r[q])
                    xq.append(xqt)

            # ---- prefetch the first main-loop weight blocks so the PE
            # can roll from gates straight into the main GEMMs ----
            wtiles = {}

            def fetch_w(db, rep=rep, wtiles=wtiles, eng=None):
                if db < NDB and db not in wtiles:
                    wt = w_p.tile([P, KT, P], BF16, tag="w",
                                  name=f"w_{rep}_{db}")
                    # rolling fetches ride the second HWDGE queue
                    # (Activation) so they never serialize against the
                    # x/out stream on sync
                    (eng or nc.scalar).dma_start(out=wt[:], in_=w_d[db])
                    wtiles[db] = wt

            # first blocks kicked on sync: their transfers queue right
            # behind the x quarters and land as the gates GEMM finishes
            for db0 in range(3):
                fetch_w(db0, eng=nc.sync)

            # ---- gates: gacc[s, b] = sum_i Wg[s,i] xT[i,b] ----
            # (gates rotate through the main acc ring; PSUM is fully given
            # to ps_m so the main loop can run 3 segment GEMMs ahead)
            gacc = ps_m.tile([P, BSL], FP32, tag="acc")
            for kt in range(KT):
                q, j = divmod(kt, JQ)
                nc.tensor.matmul(gacc[0:NSEG, 0:H], wgt[:, kt, :],
                                 xq[q][:, j, 0:H],
                                 start=(kt == 0), stop=(kt == KT - 1))
                nc.tensor.matmul(gacc[0:NSEG, H:BSL], wgt[:, kt, :],
                                 xq[q][:, j, H:BSL],
                                 start=(kt == 0), stop=(kt == KT - 1))
            gsig = mx_p.tile([NSEG, BSL], BF16, tag="gsig", bufs=1)
            nc.scalar.activation(gsig[:], gacc[0:NSEG, :], AF.Sigmoid,
                                 bias=bgp[:])
            gbc = gbc_p.tile([P, NSEG, BSL], BF16, tag="gbc")

            # ---- main loop: 64 weight blocks, dh-major ----
            for dh in range(NDH):
                sts = {}
                for s in range(NSEG):
                    db = dh * NSEG + s
                    fetch_w(db)
                    w = wtiles.pop(db)
                    acc = ps_m.tile([P, BSL], FP32, tag="acc")
                    for kt in range(KT):
                        q, j = divmod(kt, JQ)
                        for c in (0, 2, 1, 3):
                            # N=256 matmuls beat N=512 by ~6%, and visiting
                            # the two PSUM banks alternately (0,2,1,3) saves
                            # another ~6% (bank write-port settle).
                            nc.tensor.matmul(
                                acc[:, c * 256:(c + 1) * 256], w[:, kt, :],
                                xq[q][:, j, c * 256:(c + 1) * 256],
                                start=(kt == 0 and c % 2 == 0),
                                stop=(kt == KT - 1))
                    fetch_w(db + 3)
                    if dh == 0 and s == 0:
                        # gate broadcast now: gsig is ready and the PE has
                        # been kept busy by the s0 GEMM (no sigmoid stall)
                        for sg in range(NSEG):
                            gb = ps_m.tile([P, BSL], FP32, tag="acc",
                                           name=f"gb_{rep}_{sg}")
                            lhs = selt[:, sg * P:(sg + 1) * P]
                            nc.tensor.matmul(gb[:, 0:H], lhs, gsig[:, 0:H],
                                             start=True, stop=True)
                            nc.tensor.matmul(gb[:, H:BSL], lhs,
                                             gsig[:, H:BSL],
                                             start=True, stop=True)
                            nc.scalar.copy(out=gbc[:, sg, :], in_=gb[:])
                    # plateau = sigmoid(5*seg - 5*thr) = sigmoid(5*acc + c1)
                    # (last dh: run per-seg ops in halves, st on DVE, to cut
                    # the end-of-kernel drain latency)
                    lastb = dh == NDH - 1 and rep == repeats - 1
                    segcols = ((0, H), (H, BSL)) if lastb else ((0, BSL),)
                    pl = pl_p.tile([P, BSL], BF16, tag="pl")
                    u = u_p.tile([P, BSL], BF16, tag=f"u{s}")
                    st = st_p.tile([P, BSL], BF16, tag=f"st{s}")
                    for slo, shi in segcols:
                        ss = slice(slo, shi)
                        nc.scalar.activation(
                            pl[:, ss], acc[:, ss], AF.Sigmoid, scale=5.0,
                            bias=cbt[:, NDB + db:NDB + db + 1])
                        # u = (acc + b1) * plateau  (bf16 out)
                        nc.vector.scalar_tensor_tensor(
                            out=u[:, ss], in0=acc[:, ss],
                            scalar=cbt[:, db:db + 1], in1=pl[:, ss],
                            op0=ALU.add, op1=ALU.mult)
                        # st = u * gate  (pool, bf16; DVE on the last block)
                        eng = nc.vector if lastb else nc.gpsimd
                        eng.tensor_mul(st[:, ss], u[:, ss], gbc[:, s, ss])
                    sts[s] = st

                # combine; for the last dh run in two halves to cut the
                # end-of-kernel drain latency
                s01 = mx_p.tile([P, BSL], BF16, tag="s01")
                s23 = mx_p.tile([P, BSL], BF16, tag="s23")
                ssum = mx_p.tile([P, BSL], BF16, tag="ssum")
                p01 = mx_p.tile([P, BSL], BF16, tag="p01")
                p23 = mx_p.tile([P, BSL], BF16, tag="p23")
                qd = mx_p.tile([P, BSL], BF16, tag="qd")
                t1 = mx_p.tile([P, BSL], U16, tag="t1")
                sb = mx_p.tile([P, BSL], U16, tag="sb")
                t2 = mx_p.tile([P, BSL], U16, tag="t2")
                res = mx_p.tile([P, BSL], BF16, tag="res")
                last = dh == NDH - 1 and rep == repeats - 1
                cols = ((0, H), (H, BSL)) if last else ((0, BSL),)
                for lo, hi in cols:
                    cs = slice(lo, hi)
                    nc.vector.tensor_add(s01[:, cs], sts[0][:, cs],
                                         sts[1][:, cs])
                    nc.vector.tensor_add(s23[:, cs], sts[2][:, cs],
                                         sts[3][:, cs])
                    nc.vector.tensor_add(ssum[:, cs], s01[:, cs], s23[:, cs])
                    nc.vector.tensor_mul(p01[:, cs], sts[0][:, cs],
                                         sts[1][:, cs])
                    # p23 = (st2 * 1e-4) * st3  (folds 0.1^4 of the final
                    # 0.1*|prod|^(1/4) term into the product)
                    nc.vector.scalar_tensor_tensor(
                        out=p23[:, cs], in0=sts[2][:, cs], scalar=1e-4,
                        in1=sts[3][:, cs], op0=ALU.mult, op1=ALU.mult)
                    nc.vector.tensor_mul(qd[:, cs], p01[:, cs], p23[:, cs])
                    # t2 = sign(q) * |q|^(1/4) via bf16-bits magic on DVE
                    qi = qd[:, cs].bitcast(U16)
                    nc.vector.tensor_scalar(
                        out=t1[:, cs], in0=qi, scalar1=2, scalar2=0x1FFF,
                        op0=ALU.logical_shift_right, op1=ALU.bitwise_and)
                    nc.vector.tensor_scalar(
                        out=sb[:, cs], in0=qi, scalar1=0x8000, scalar2=None,
                        op0=ALU.bitwise_and)
                    # (t1 + MAGIC) has bit15 clear, so "| signbit" == "+"
                    nc.vector.scalar_tensor_tensor(
                        out=t2[:, cs], in0=t1[:, cs], scalar=MAGIC,
                        in1=sb[:, cs], op0=ALU.add, op1=ALU.add)
                    nc.vector.tensor_add(res[:, cs], ssum[:, cs],
                                         t2[:, cs].bitcast(BF16))
                    nc.sync.dma_start(out=out_r[dh][:, cs], in_=res[:, cs])

    nc.compile()
    return nc


_NC_CACHE = {}


def _get_nc():
    if "nc" not in _NC_CACHE:
        _NC_CACHE["nc"] = build_nc()
    return _NC_CACHE["nc"]


def make_in_maps(x, W_seg, b_seg, threshold, W_gate, b_gate):
    x = np.asarray(x, dtype=np.float32)
    xT = np.ascontiguousarray(x.T.astype(NPBF))                     # [2048, 8192]

    # W blocks: w[db=dh*4+s, ii, kt, dd] = W_seg[s, dh*128+dd, kt*128+ii]
    Wb = np.asarray(W_seg, dtype=np.float32).reshape(NSEG, NDH, P, KT, P)
    w_arr = np.ascontiguousarray(
        Wb.transpose(1, 0, 4, 3, 2).astype(NPBF)).reshape(NDB, P, KT, P)

    # wg[p, kt*4+s] = W_gate[s, kt*128+p]
    wg_arr = np.ascontiguousarray(
        np.asarray(W_gate, dtype=np.float32).T.reshape(KT, P, NSEG)
        .transpose(1, 0, 2).astype(NPBF)).reshape(P, KT * NSEG)

    sel = np.zeros((NSEG, NSEG * P), dtype=NPBF)
    for s in range(NSEG):
        sel[s, s * P:(s + 1) * P] = 1.0

    # cb[:, db] = b1 column; cb[:, NDB+db] = 5*(b1 - thr) column
    bs = np.asarray(b_seg, dtype=np.float32).reshape(NSEG, NDH, P)
    th = np.asarray(threshold, dtype=np.float32).reshape(NSEG, NDH, P)
    cb = np.empty((P, 2 * NDB), dtype=np.float32)
    cb[:, :NDB] = bs.transpose(2, 1, 0).reshape(P, NDB)
    cb[:, NDB:] = (5.0 * (bs - th)).transpose(2, 1, 0).reshape(P, NDB)

    bg = np.asarray(b_gate, dtype=np.float32)

    in_maps = []
    for c in range(NCORES):
        in_maps.append({
            "xt": np.ascontiguousarray(xT[:, c * BSL:(c + 1) * BSL]),
            "w": w_arr,
            "wg": wg_arr,
            "bg": bg,
            "sel": sel,
            "cb": cb,
        })
    return in_maps


def kernel(x, W_seg, b_seg, threshold, W_gate, b_gate):
    nc = _get_nc()
    in_maps = make_in_maps(x, W_seg, b_seg, threshold, W_gate, b_gate)
    res = run_bass_kernel_spmd(nc, in_maps, core_ids=list(range(NCORES)))
    return np.concatenate(
        [res.results[c]["out"].astype(np.float32).T for c in range(NCORES)],
        axis=0)
